# revision 6
# baseline (speedup 1.0000x reference)
"""Trainium2 Bass kernel for the EntropyResidualBlock (two masked 5x5 convs,
PReLU, residual) on 8 NeuronCores.

Sharding: 8 cores = 2 batches x 4 H-strips of 64 rows. Each core recomputes a
2-row y1 halo at the top of its strip (no cross-core communication); x halos
are host-zero-padded and a per-core halo-mask input zeroes the y1 halo rows
for strips at the image top (so conv2 sees correct zero padding).

The PixelCNN mask leaves only 13 of 25 taps nonzero (rows above center + left
of center + group-causal center), so each conv row is 13 taps x 3 ci-chunks x
3 co-chunks = 117 bf16 matmuls of [128,128]x[128,512] accumulated in PSUM.
conv1 -> conv2 are fused through rolling 4-row SBUF ring buffers.
"""

import os
import sys

import numpy as np
import ml_dtypes

for _p in ("/opt/trn_rl_repo",):
    if os.path.isdir(_p) and _p not in sys.path:
        sys.path.append(_p)

import concourse.bass as bass  # noqa: E402
import concourse.tile as tile  # noqa: E402
from concourse import bacc, mybir  # noqa: E402
from concourse.bass import ds  # noqa: E402
from concourse.bass_utils import run_bass_kernel_spmd  # noqa: E402

BF16NP = ml_dtypes.bfloat16
F32 = mybir.dt.float32
BF16 = mybir.dt.bfloat16
AF = mybir.ActivationFunctionType

B, C, H, W = 2, 384, 256, 512
NG, CPN, KS, PAD = 16, 24, 5, 2
NCORES = 8
SPB = 4            # strips per batch
HS = H // SPB      # 64 output rows per core
WP = 520           # padded row width in SBUF/DRAM (2 left pad + 512 + 6)
NR = HS + 5        # x rows staged per core: r0-4 .. r0+64 (last is prefetch slack)
TAPS = [(kh, kw) for kh in (0, 1) for kw in range(KS)] + [(2, 0), (2, 1), (2, 2)]
NT = len(TAPS)     # 13
NMM = NT * 3       # matmuls per psum accumulation group


def _build_mask() -> np.ndarray:
    m = np.zeros((C, C, KS, KS), np.float32)
    m[:, :, :PAD, :] = 1.0
    m[:, :, PAD, :PAD] = 1.0
    g = np.arange(C) // CPN
    m[:, :, PAD, PAD] = (g[None, :] <= g[:, None]).astype(np.float32)
    return m


def _build_nc():
    nc = bacc.Bacc("TRN2", target_bir_lowering=False, debug=False,
                   num_devices=NCORES)
    xs_d = nc.dram_tensor("xs", [128, NR * 3, WP], F32, kind="ExternalInput").ap()
    w1_d = nc.dram_tensor("w1t", [128, NT, 3, 384], BF16, kind="ExternalInput").ap()
    w2_d = nc.dram_tensor("w2t", [128, NT, 3, 384], BF16, kind="ExternalInput").ap()
    b1_d = nc.dram_tensor("b1c", [128, 3], F32, kind="ExternalInput").ap()
    a1_d = nc.dram_tensor("a1c", [128, 3], F32, kind="ExternalInput").ap()
    b2_d = nc.dram_tensor("b2c", [128, 3], F32, kind="ExternalInput").ap()
    a2_d = nc.dram_tensor("a2c", [128, 3], F32, kind="ExternalInput").ap()
    hm_d = nc.dram_tensor("hm", [128, 2], F32, kind="ExternalInput").ap()
    ys_d = nc.dram_tensor("ys", [128, HS * 3, W], F32, kind="ExternalOutput").ap()

    with tile.TileContext(nc) as tc:
        with tc.tile_pool(name="wp", bufs=1) as wp, \
             tc.tile_pool(name="cp", bufs=1) as cp, \
             tc.tile_pool(name="ring", bufs=1) as rp, \
             tc.tile_pool(name="op", bufs=4) as op, \
             tc.tile_pool(name="pp", bufs=8, space="PSUM") as pp:

            w1t = wp.tile([128, NT, 3, 384], BF16, name="w1sb", tag="w1sb")
            nc.sync.dma_start(out=w1t, in_=w1_d)
            w2t = wp.tile([128, NT, 3, 384], BF16, name="w2sb", tag="w2sb")
            nc.sync.dma_start(out=w2t, in_=w2_d)
            b1c = cp.tile([128, 3], F32, name="b1sb", tag="b1sb")
            nc.sync.dma_start(out=b1c, in_=b1_d)
            a1c = cp.tile([128, 3], F32, name="a1sb", tag="a1sb")
            nc.sync.dma_start(out=a1c, in_=a1_d)
            b2c = cp.tile([128, 3], F32, name="b2sb", tag="b2sb")
            nc.sync.dma_start(out=b2c, in_=b2_d)
            a2c = cp.tile([128, 3], F32, name="a2sb", tag="a2sb")
            nc.sync.dma_start(out=a2c, in_=a2_d)
            hm = cp.tile([128, 2], F32, name="hmsb", tag="hmsb")
            nc.sync.dma_start(out=hm, in_=hm_d)

            xf = [rp.tile([128, 3, WP], F32, name=f"xf{j}", tag=f"xf{j}")
                  for j in range(4)]
            xb = [rp.tile([128, 3, WP], BF16, name=f"xb{j}", tag=f"xb{j}")
                  for j in range(4)]
            y1 = [rp.tile([128, 3, WP], BF16, name=f"y1r{j}", tag=f"y1r{j}")
                  for j in range(4)]
            for j in range(4):
                # W-pad columns of the y1 ring stay zero forever (conv2 zero pad)
                nc.vector.memset(y1[j][:, :, 0:PAD], 0.0)
                nc.vector.memset(y1[j][:, :, PAD + W:WP], 0.0)

            def load_x_row(row_expr, slot):
                # one DMA: [128, 3, WP] fp32 row (host zero-padded), then bf16 cast
                nc.sync.dma_start(out=xf[slot], in_=xs_d[:, ds(row_expr * 3, 3), :])
                nc.vector.tensor_copy(xb[slot], xf[slot])

            def conv_row(wt, ring, hmod, epilogue, dh0_last=False):
                taps = sorted(TAPS, key=lambda t: t[0] == 2) if dh0_last else TAPS
                for coc in range(3):
                    ps = pp.tile([128, W], F32, name="ps", tag="ps")
                    work = [(kh, kw, cic) for (kh, kw) in taps for cic in range(3)
                            # group-causal center tap: ci chunk 2 (groups>=10)
                            # never feeds co chunk 0 (groups<=5) - weights all 0
                            if not (kh == 2 and kw == 2 and cic == 2 and coc == 0)]
                    for n, (kh, kw, cic) in enumerate(work):
                        dh, dw = kh - 2, kw - 2
                        src = ring[(hmod + dh) % 4]
                        ti = TAPS.index((kh, kw))
                        nc.tensor.matmul(
                            ps,
                            lhsT=wt[:, ti, cic, coc * 128:(coc + 1) * 128],
                            rhs=src[:, cic, PAD + dw: PAD + dw + W],
                            start=(n == 0), stop=(n == len(work) - 1))
                    epilogue(coc, ps)

            def y1_epilogue(slot):
                def f(coc, ps):
                    nc.scalar.activation(
                        y1[slot][:, coc, PAD:PAD + W], ps, AF.Prelu,
                        bias=b1c[:, coc:coc + 1], scale=1.0,
                        alpha=a1c[:, coc:coc + 1])
                return f

            # ---- prologue: x[-4..-1], then y1[-2], y1[-1] (halo, maskable) ----
            for j in range(4):
                load_x_row(j, j)          # xs row j = x[r0-4+j] -> slot j
            conv_row(w1t, xb, (-2) % 4, y1_epilogue((-2) % 4))
            nc.vector.tensor_scalar_mul(y1[(-2) % 4], y1[(-2) % 4], hm[:, 0:1])
            load_x_row(4, 0)              # x[0] -> slot 0
            conv_row(w1t, xb, (-1) % 4, y1_epilogue((-1) % 4))
            nc.vector.tensor_scalar_mul(y1[(-1) % 4], y1[(-1) % 4], hm[:, 1:2])

            # ---- main loop: 8 groups x 8 rows (ring slots = j%4, static) ----
            with tc.For_i(0, HS // 8, 1,
                          hint_engines=(mybir.EngineType.PE,),
                          staggered_reset=True) as i:
                for j in range(8):
                    # row h = 8*i + j; prefetch x[h+1] (xs row h+5)
                    load_x_row(i * 8 + (j + 5), (j + 1) % 4)
                    conv_row(w1t, xb, j % 4, y1_epilogue(j % 4))

                    y2s = op.tile([128, 3, W], F32, name="y2s", tag="y2s")

                    def y2_epilogue(coc, ps, j=j, y2s=y2s):
                        nc.scalar.activation(
                            y2s[:, coc, :], ps, AF.Prelu,
                            bias=b2c[:, coc:coc + 1], scale=1.0,
                            alpha=a2c[:, coc:coc + 1])
                        nc.vector.tensor_add(
                            y2s[:, coc, :], y2s[:, coc, :],
                            xf[j % 4][:, coc, PAD:PAD + W])

                    conv_row(w2t, y1, j % 4, y2_epilogue, dh0_last=True)
                    nc.sync.dma_start(out=ys_d[:, ds((i * 8 + j) * 3, 3), :],
                                      in_=y2s)

    nc.compile()
    return nc


_NC_CACHE = {}


def _get_nc():
    if "nc" not in _NC_CACHE:
        _NC_CACHE["nc"] = _build_nc()
    return _NC_CACHE["nc"]


def kernel(x, w1, b1, a1, w2, b2, a2, _trace_dir=None, _trace_cores=None):
    x = np.asarray(x, np.float32)
    mask = _build_mask()
    w1m = np.asarray(w1, np.float32) * mask
    w2m = np.asarray(w2, np.float32) * mask

    # lhsT weight layout: [ci_mod(p), tap, ci_chunk, co], bf16
    def wT(wm):
        wr = wm.reshape(C, 3, 128, KS, KS)          # [o, c, p, kh, kw]
        out = np.empty((128, NT, 3, C), np.float32)
        for t, (kh, kw) in enumerate(TAPS):
            out[:, t, :, :] = wr[:, :, :, kh, kw].transpose(2, 1, 0)
        return np.ascontiguousarray(out.astype(BF16NP))

    w1t_np, w2t_np = wT(w1m), wT(w2m)

    def chunked(v):  # [384] -> [128, 3]
        return np.ascontiguousarray(np.asarray(v, np.float32).reshape(3, 128).T)

    b1c, a1c = chunked(b1), chunked(a1)
    b2c, a2c = chunked(b2), chunked(a2)

    xq = x.reshape(B, 3, 128, H, W)
    in_maps = []
    for core in range(NCORES):
        b_, s = divmod(core, SPB)
        r0 = s * HS
        xs = np.zeros((128, NR, 3, WP), np.float32)
        lo, hi = r0 - 4, r0 - 4 + NR          # global rows [lo, hi)
        glo, ghi = max(lo, 0), min(hi, H)
        if ghi > glo:
            xs[:, glo - lo:ghi - lo, :, PAD:PAD + W] = \
                xq[b_, :, :, glo:ghi, :].transpose(1, 2, 0, 3)
        hmv = np.zeros((128, 2), np.float32) if s == 0 else np.ones((128, 2), np.float32)
        in_maps.append({
            "xs": xs.reshape(128, NR * 3, WP),
            "w1t": w1t_np, "w2t": w2t_np,
            "b1c": b1c, "a1c": a1c, "b2c": b2c, "a2c": a2c,
            "hm": hmv,
        })

    nc = _get_nc()
    kw = {}
    if _trace_dir is not None:
        kw = dict(trace=True, tmpdir=_trace_dir,
                  trace_cores=_trace_cores or [0])
    res = None
    for attempt in range(3):
        try:
            res = run_bass_kernel_spmd(nc, in_maps,
                                       core_ids=list(range(NCORES)), **kw)
            break
        except Exception:
            # transient NRT/axon device errors recover on retry
            if attempt == 2:
                raise
            import time
            time.sleep(5)

    y = np.empty_like(x)
    for core in range(NCORES):
        b_, s = divmod(core, SPB)
        r0 = s * HS
        ys = res.results[core]["ys"].reshape(128, HS, 3, W)
        y[b_, :, r0:r0 + HS, :] = ys.transpose(2, 0, 1, 3).reshape(C, HS, W)
    if _trace_dir is not None:
        return y, res
    return y


# revision 7
# speedup vs baseline: 1.1881x; 1.1881x over previous
"""Trainium2 Bass kernel for the EntropyResidualBlock (two masked 5x5 convs,
PReLU, residual) on 8 NeuronCores.

Sharding: 8 cores = 2 batches x 4 H-strips of 64 rows. Each core recomputes a
2-row y1 halo at the top of its strip (no cross-core communication); x halos
are host-zero-padded and a per-core halo-mask input zeroes the y1 halo rows
for strips at the image top (so conv2 sees correct zero padding).

The PixelCNN mask leaves only 13 of 25 taps nonzero (rows above center + left
of center + group-causal center), so each conv row is 13 taps x 3 ci-chunks x
3 co-chunks = 117 bf16 matmuls of [128,128]x[128,512] accumulated in PSUM.
conv1 -> conv2 are fused through rolling 4-row SBUF ring buffers.
"""

import os
import sys

import numpy as np
import ml_dtypes

for _p in ("/opt/trn_rl_repo",):
    if os.path.isdir(_p) and _p not in sys.path:
        sys.path.append(_p)

import concourse.bass as bass  # noqa: E402
import concourse.tile as tile  # noqa: E402
from concourse import bacc, mybir  # noqa: E402
from concourse.bass import ds  # noqa: E402
from concourse.bass_utils import run_bass_kernel_spmd  # noqa: E402

BF16NP = ml_dtypes.bfloat16
F32 = mybir.dt.float32
BF16 = mybir.dt.bfloat16
AF = mybir.ActivationFunctionType

B, C, H, W = 2, 384, 256, 512
NG, CPN, KS, PAD = 16, 24, 5, 2
NCORES = 8
SPB = 4            # strips per batch
HS = H // SPB      # 64 output rows per core
WP = 520           # padded row width in SBUF/DRAM (2 left pad + 512 + 6)
NR = HS + 5        # x rows staged per core: r0-4 .. r0+64 (last is prefetch slack)
TAPS = [(kh, kw) for kh in (0, 1) for kw in range(KS)] + [(2, 0), (2, 1), (2, 2)]
NT = len(TAPS)     # 13
NMM = NT * 3       # matmuls per psum accumulation group


def _build_mask() -> np.ndarray:
    m = np.zeros((C, C, KS, KS), np.float32)
    m[:, :, :PAD, :] = 1.0
    m[:, :, PAD, :PAD] = 1.0
    g = np.arange(C) // CPN
    m[:, :, PAD, PAD] = (g[None, :] <= g[:, None]).astype(np.float32)
    return m


def _build_nc():
    nc = bacc.Bacc("TRN2", target_bir_lowering=False, debug=False,
                   num_devices=NCORES)
    xs_d = nc.dram_tensor("xs", [128, NR * 3, WP], F32, kind="ExternalInput").ap()
    w1_d = nc.dram_tensor("w1t", [128, NT, 3, 384], BF16, kind="ExternalInput").ap()
    w2_d = nc.dram_tensor("w2t", [128, NT, 3, 384], BF16, kind="ExternalInput").ap()
    b1_d = nc.dram_tensor("b1c", [128, 3], F32, kind="ExternalInput").ap()
    a1_d = nc.dram_tensor("a1c", [128, 3], F32, kind="ExternalInput").ap()
    b2_d = nc.dram_tensor("b2c", [128, 3], F32, kind="ExternalInput").ap()
    a2_d = nc.dram_tensor("a2c", [128, 3], F32, kind="ExternalInput").ap()
    hm_d = nc.dram_tensor("hm", [128, 2], F32, kind="ExternalInput").ap()
    ys_d = nc.dram_tensor("ys", [128, HS * 3, W], F32, kind="ExternalOutput").ap()

    with tile.TileContext(nc) as tc:
        with tc.tile_pool(name="wp", bufs=1) as wp, \
             tc.tile_pool(name="cp", bufs=1) as cp, \
             tc.tile_pool(name="ring", bufs=1) as rp, \
             tc.tile_pool(name="op", bufs=4) as op, \
             tc.tile_pool(name="pp", bufs=8, space="PSUM") as pp:

            w1t = wp.tile([128, NT, 3, 384], BF16, name="w1sb", tag="w1sb")
            nc.sync.dma_start(out=w1t, in_=w1_d)
            w2t = wp.tile([128, NT, 3, 384], BF16, name="w2sb", tag="w2sb")
            nc.sync.dma_start(out=w2t, in_=w2_d)
            b1c = cp.tile([128, 3], F32, name="b1sb", tag="b1sb")
            nc.sync.dma_start(out=b1c, in_=b1_d)
            a1c = cp.tile([128, 3], F32, name="a1sb", tag="a1sb")
            nc.sync.dma_start(out=a1c, in_=a1_d)
            b2c = cp.tile([128, 3], F32, name="b2sb", tag="b2sb")
            nc.sync.dma_start(out=b2c, in_=b2_d)
            a2c = cp.tile([128, 3], F32, name="a2sb", tag="a2sb")
            nc.sync.dma_start(out=a2c, in_=a2_d)
            hm = cp.tile([128, 2], F32, name="hmsb", tag="hmsb")
            nc.sync.dma_start(out=hm, in_=hm_d)

            xf = [rp.tile([128, 3, WP], F32, name=f"xf{j}", tag=f"xf{j}")
                  for j in range(4)]
            xb = [rp.tile([128, 3, WP], BF16, name=f"xb{j}", tag=f"xb{j}")
                  for j in range(4)]
            y1 = [rp.tile([128, 3, WP], BF16, name=f"y1r{j}", tag=f"y1r{j}")
                  for j in range(4)]
            for j in range(4):
                # W-pad columns of the y1 ring stay zero forever (conv2 zero pad)
                nc.vector.memset(y1[j][:, :, 0:PAD], 0.0)
                nc.vector.memset(y1[j][:, :, PAD + W:WP], 0.0)

            def load_x_row(row_expr, slot):
                # one DMA: [128, 3, WP] fp32 row (host zero-padded), then bf16 cast
                nc.sync.dma_start(out=xf[slot], in_=xs_d[:, ds(row_expr * 3, 3), :])
                nc.vector.tensor_copy(xb[slot], xf[slot])

            def conv_row(wt, ring, hmod, epilogue, dh0_last=False):
                taps = sorted(TAPS, key=lambda t: t[0] == 2) if dh0_last else TAPS
                for coc in range(3):
                    ps = pp.tile([128, W], F32, name="ps", tag="ps")
                    work = [(kh, kw, cic) for (kh, kw) in taps for cic in range(3)
                            # group-causal center tap: ci chunk 2 (groups>=10)
                            # never feeds co chunk 0 (groups<=5) - weights all 0
                            if not (kh == 2 and kw == 2 and cic == 2 and coc == 0)]
                    for n, (kh, kw, cic) in enumerate(work):
                        dh, dw = kh - 2, kw - 2
                        src = ring[(hmod + dh) % 4]
                        ti = TAPS.index((kh, kw))
                        nc.tensor.matmul(
                            ps,
                            lhsT=wt[:, ti, cic, coc * 128:(coc + 1) * 128],
                            rhs=src[:, cic, PAD + dw: PAD + dw + W],
                            start=(n == 0), stop=(n == len(work) - 1))
                    epilogue(coc, ps)

            def y1_epilogue(slot):
                def f(coc, ps):
                    nc.scalar.activation(
                        y1[slot][:, coc, PAD:PAD + W], ps, AF.Prelu,
                        bias=b1c[:, coc:coc + 1], scale=1.0,
                        alpha=a1c[:, coc:coc + 1])
                return f

            # ---- prologue: x[-4..-1], then y1[-2], y1[-1] (halo, maskable) ----
            for j in range(4):
                load_x_row(j, j)          # xs row j = x[r0-4+j] -> slot j
            conv_row(w1t, xb, (-2) % 4, y1_epilogue((-2) % 4))
            nc.vector.tensor_scalar_mul(y1[(-2) % 4], y1[(-2) % 4], hm[:, 0:1])
            load_x_row(4, 0)              # x[0] -> slot 0
            conv_row(w1t, xb, (-1) % 4, y1_epilogue((-1) % 4))
            nc.vector.tensor_scalar_mul(y1[(-1) % 4], y1[(-1) % 4], hm[:, 1:2])

            # ---- main loop: 8 groups x 8 rows (ring slots = j%4, static) ----
            with tc.For_i(0, HS // 8, 1,
                          hint_engines=(mybir.EngineType.PE,)) as i:
                for j in range(8):
                    # row h = 8*i + j; prefetch x[h+1] (xs row h+5)
                    load_x_row(i * 8 + (j + 5), (j + 1) % 4)
                    conv_row(w1t, xb, j % 4, y1_epilogue(j % 4))

                    y2s = op.tile([128, 3, W], F32, name="y2s", tag="y2s")

                    def y2_epilogue(coc, ps, j=j, y2s=y2s):
                        nc.scalar.activation(
                            y2s[:, coc, :], ps, AF.Prelu,
                            bias=b2c[:, coc:coc + 1], scale=1.0,
                            alpha=a2c[:, coc:coc + 1])
                        nc.vector.tensor_add(
                            y2s[:, coc, :], y2s[:, coc, :],
                            xf[j % 4][:, coc, PAD:PAD + W])

                    conv_row(w2t, y1, j % 4, y2_epilogue, dh0_last=True)
                    nc.sync.dma_start(out=ys_d[:, ds((i * 8 + j) * 3, 3), :],
                                      in_=y2s)

    nc.compile()
    return nc


_NC_CACHE = {}


def _get_nc():
    if "nc" not in _NC_CACHE:
        _NC_CACHE["nc"] = _build_nc()
    return _NC_CACHE["nc"]


def kernel(x, w1, b1, a1, w2, b2, a2, _trace_dir=None, _trace_cores=None):
    x = np.asarray(x, np.float32)
    mask = _build_mask()
    w1m = np.asarray(w1, np.float32) * mask
    w2m = np.asarray(w2, np.float32) * mask

    # lhsT weight layout: [ci_mod(p), tap, ci_chunk, co], bf16
    def wT(wm):
        wr = wm.reshape(C, 3, 128, KS, KS)          # [o, c, p, kh, kw]
        out = np.empty((128, NT, 3, C), np.float32)
        for t, (kh, kw) in enumerate(TAPS):
            out[:, t, :, :] = wr[:, :, :, kh, kw].transpose(2, 1, 0)
        return np.ascontiguousarray(out.astype(BF16NP))

    w1t_np, w2t_np = wT(w1m), wT(w2m)

    def chunked(v):  # [384] -> [128, 3]
        return np.ascontiguousarray(np.asarray(v, np.float32).reshape(3, 128).T)

    b1c, a1c = chunked(b1), chunked(a1)
    b2c, a2c = chunked(b2), chunked(a2)

    xq = x.reshape(B, 3, 128, H, W)
    in_maps = []
    for core in range(NCORES):
        b_, s = divmod(core, SPB)
        r0 = s * HS
        xs = np.zeros((128, NR, 3, WP), np.float32)
        lo, hi = r0 - 4, r0 - 4 + NR          # global rows [lo, hi)
        glo, ghi = max(lo, 0), min(hi, H)
        if ghi > glo:
            xs[:, glo - lo:ghi - lo, :, PAD:PAD + W] = \
                xq[b_, :, :, glo:ghi, :].transpose(1, 2, 0, 3)
        hmv = np.zeros((128, 2), np.float32) if s == 0 else np.ones((128, 2), np.float32)
        in_maps.append({
            "xs": xs.reshape(128, NR * 3, WP),
            "w1t": w1t_np, "w2t": w2t_np,
            "b1c": b1c, "a1c": a1c, "b2c": b2c, "a2c": a2c,
            "hm": hmv,
        })

    nc = _get_nc()
    kw = {}
    if _trace_dir is not None:
        kw = dict(trace=True, tmpdir=_trace_dir,
                  trace_cores=_trace_cores or [0])
    res = None
    for attempt in range(3):
        try:
            res = run_bass_kernel_spmd(nc, in_maps,
                                       core_ids=list(range(NCORES)), **kw)
            break
        except Exception:
            # transient NRT/axon device errors recover on retry
            if attempt == 2:
                raise
            import time
            time.sleep(5)

    y = np.empty_like(x)
    for core in range(NCORES):
        b_, s = divmod(core, SPB)
        r0 = s * HS
        ys = res.results[core]["ys"].reshape(128, HS, 3, W)
        y[b_, :, r0:r0 + HS, :] = ys.transpose(2, 0, 1, 3).reshape(C, HS, W)
    if _trace_dir is not None:
        return y, res
    return y


# revision 11
# speedup vs baseline: 1.1921x; 1.0033x over previous
"""Trainium2 Bass kernel for the EntropyResidualBlock (two masked 5x5 convs,
PReLU, residual) on 8 NeuronCores.

Sharding: 8 cores = 2 batches x 4 H-strips of 64 rows. Each core recomputes a
2-row y1 halo at the top of its strip (no cross-core communication); x halos
are host-zero-padded and a per-core halo-mask input zeroes the y1 halo rows
for strips at the image top (so conv2 sees correct zero padding).

The PixelCNN mask leaves only 13 of 25 taps nonzero (rows above center + left
of center + group-causal center), so each conv row is 13 taps x 3 ci-chunks x
3 co-chunks = 117 bf16 matmuls of [128,128]x[128,512] accumulated in PSUM.
conv1 -> conv2 are fused through rolling 4-row SBUF ring buffers.
"""

import os
import sys

import numpy as np
import ml_dtypes

for _p in ("/opt/trn_rl_repo",):
    if os.path.isdir(_p) and _p not in sys.path:
        sys.path.append(_p)

import concourse.bass as bass  # noqa: E402
import concourse.tile as tile  # noqa: E402
from concourse import bacc, mybir  # noqa: E402
from concourse.bass import ds  # noqa: E402
from concourse.bass_utils import run_bass_kernel_spmd  # noqa: E402

BF16NP = ml_dtypes.bfloat16
F32 = mybir.dt.float32
BF16 = mybir.dt.bfloat16
AF = mybir.ActivationFunctionType

B, C, H, W = 2, 384, 256, 512
NG, CPN, KS, PAD = 16, 24, 5, 2
NCORES = 8
SPB = 4            # strips per batch
HS = H // SPB      # 64 output rows per core
WP = 520           # padded row width in SBUF/DRAM (2 left pad + 512 + 6)
NR = HS + 5        # x rows staged per core: r0-4 .. r0+64 (last is prefetch slack)
TAPS = [(kh, kw) for kh in (0, 1) for kw in range(KS)] + [(2, 0), (2, 1), (2, 2)]
NT = len(TAPS)     # 13
NMM = NT * 3       # matmuls per psum accumulation group


def _build_mask() -> np.ndarray:
    m = np.zeros((C, C, KS, KS), np.float32)
    m[:, :, :PAD, :] = 1.0
    m[:, :, PAD, :PAD] = 1.0
    g = np.arange(C) // CPN
    m[:, :, PAD, PAD] = (g[None, :] <= g[:, None]).astype(np.float32)
    return m


def _build_nc():
    nc = bacc.Bacc("TRN2", target_bir_lowering=False, debug=False,
                   num_devices=NCORES)
    xs_d = nc.dram_tensor("xs", [128, NR * 3, WP], F32, kind="ExternalInput").ap()
    w1_d = nc.dram_tensor("w1t", [128, NT, 3, 384], BF16, kind="ExternalInput").ap()
    w2_d = nc.dram_tensor("w2t", [128, NT, 3, 384], BF16, kind="ExternalInput").ap()
    b1_d = nc.dram_tensor("b1c", [128, 3], F32, kind="ExternalInput").ap()
    a1_d = nc.dram_tensor("a1c", [128, 3], F32, kind="ExternalInput").ap()
    b2_d = nc.dram_tensor("b2c", [128, 3], F32, kind="ExternalInput").ap()
    a2_d = nc.dram_tensor("a2c", [128, 3], F32, kind="ExternalInput").ap()
    hm_d = nc.dram_tensor("hm", [128, 2], F32, kind="ExternalInput").ap()
    ys_d = nc.dram_tensor("ys", [128, HS * 3, W], F32, kind="ExternalOutput").ap()

    with tile.TileContext(nc) as tc:
        with tc.tile_pool(name="wp", bufs=1) as wp, \
             tc.tile_pool(name="cp", bufs=1) as cp, \
             tc.tile_pool(name="ring", bufs=1) as rp, \
             tc.tile_pool(name="op", bufs=4) as op, \
             tc.tile_pool(name="pp", bufs=8, space="PSUM") as pp:

            w1t = wp.tile([128, NT, 3, 384], BF16, name="w1sb", tag="w1sb")
            nc.gpsimd.dma_start(out=w1t, in_=w1_d)
            b1c = cp.tile([128, 3], F32, name="b1sb", tag="b1sb")
            nc.gpsimd.dma_start(out=b1c, in_=b1_d)
            a1c = cp.tile([128, 3], F32, name="a1sb", tag="a1sb")
            nc.gpsimd.dma_start(out=a1c, in_=a1_d)
            b2c = cp.tile([128, 3], F32, name="b2sb", tag="b2sb")
            nc.gpsimd.dma_start(out=b2c, in_=b2_d)
            a2c = cp.tile([128, 3], F32, name="a2sb", tag="a2sb")
            nc.gpsimd.dma_start(out=a2c, in_=a2_d)
            hm = cp.tile([128, 2], F32, name="hmsb", tag="hmsb")
            nc.gpsimd.dma_start(out=hm, in_=hm_d)

            xf = [rp.tile([128, 3, WP], F32, name=f"xf{j}", tag=f"xf{j}")
                  for j in range(4)]
            xb = [rp.tile([128, 3, WP], BF16, name=f"xb{j}", tag=f"xb{j}")
                  for j in range(4)]
            y1 = [rp.tile([128, 3, WP], BF16, name=f"y1r{j}", tag=f"y1r{j}")
                  for j in range(4)]
            for j in range(4):
                # W-pad columns of the y1 ring stay zero forever (conv2 zero pad)
                nc.vector.memset(y1[j][:, :, 0:PAD], 0.0)
                nc.vector.memset(y1[j][:, :, PAD + W:WP], 0.0)

            def load_x_row(row_expr, slot):
                # one DMA: [128, 3, WP] fp32 row (host zero-padded), then bf16 cast
                nc.sync.dma_start(out=xf[slot], in_=xs_d[:, ds(row_expr * 3, 3), :])
                nc.vector.tensor_copy(xb[slot], xf[slot])

            def conv_row(wt, ring, hmod, epilogue, dh0_last=False):
                taps = sorted(TAPS, key=lambda t: t[0] == 2) if dh0_last else TAPS
                for coc in range(3):
                    ps = pp.tile([128, W], F32, name="ps", tag="ps")
                    work = [(kh, kw, cic) for (kh, kw) in taps for cic in range(3)
                            # group-causal center tap: ci chunk 2 (groups>=10)
                            # never feeds co chunk 0 (groups<=5) - weights all 0
                            if not (kh == 2 and kw == 2 and cic == 2 and coc == 0)]
                    for n, (kh, kw, cic) in enumerate(work):
                        dh, dw = kh - 2, kw - 2
                        src = ring[(hmod + dh) % 4]
                        ti = TAPS.index((kh, kw))
                        nc.tensor.matmul(
                            ps,
                            lhsT=wt[:, ti, cic, coc * 128:(coc + 1) * 128],
                            rhs=src[:, cic, PAD + dw: PAD + dw + W],
                            start=(n == 0), stop=(n == len(work) - 1))
                    epilogue(coc, ps)

            def y1_epilogue(slot):
                def f(coc, ps):
                    nc.scalar.activation(
                        y1[slot][:, coc, PAD:PAD + W], ps, AF.Prelu,
                        bias=b1c[:, coc:coc + 1], scale=1.0,
                        alpha=a1c[:, coc:coc + 1])
                return f

            # ---- prologue: x[-4..-1], then y1[-2], y1[-1] (halo, maskable) ----
            for j in range(4):
                load_x_row(j, j)          # xs row j = x[r0-4+j] -> slot j
            w2t = wp.tile([128, NT, 3, 384], BF16, name="w2sb", tag="w2sb")
            nc.gpsimd.dma_start(out=w2t, in_=w2_d)
            conv_row(w1t, xb, (-2) % 4, y1_epilogue((-2) % 4))
            nc.vector.tensor_scalar_mul(y1[(-2) % 4], y1[(-2) % 4], hm[:, 0:1])
            load_x_row(4, 0)              # x[0] -> slot 0
            conv_row(w1t, xb, (-1) % 4, y1_epilogue((-1) % 4))
            nc.vector.tensor_scalar_mul(y1[(-1) % 4], y1[(-1) % 4], hm[:, 1:2])

            # ---- main loop: 8 groups x 8 rows (ring slots = j%4, static) ----
            with tc.For_i(0, HS // 8, 1,
                          hint_engines=(mybir.EngineType.PE,)) as i:
                for j in range(8):
                    # row h = 8*i + j; prefetch x[h+1] (xs row h+5)
                    load_x_row(i * 8 + (j + 5), (j + 1) % 4)
                    conv_row(w1t, xb, j % 4, y1_epilogue(j % 4))

                    y2s = op.tile([128, 3, W], F32, name="y2s", tag="y2s")

                    def y2_epilogue(coc, ps, j=j, y2s=y2s):
                        nc.scalar.activation(
                            y2s[:, coc, :], ps, AF.Prelu,
                            bias=b2c[:, coc:coc + 1], scale=1.0,
                            alpha=a2c[:, coc:coc + 1])
                        nc.vector.tensor_add(
                            y2s[:, coc, :], y2s[:, coc, :],
                            xf[j % 4][:, coc, PAD:PAD + W])

                    conv_row(w2t, y1, j % 4, y2_epilogue, dh0_last=True)
                    nc.sync.dma_start(out=ys_d[:, ds((i * 8 + j) * 3, 3), :],
                                      in_=y2s)

    nc.compile()
    return nc


_NC_CACHE = {}


def _get_nc():
    if "nc" not in _NC_CACHE:
        _NC_CACHE["nc"] = _build_nc()
    return _NC_CACHE["nc"]


def kernel(x, w1, b1, a1, w2, b2, a2, _trace_dir=None, _trace_cores=None):
    x = np.asarray(x, np.float32)
    mask = _build_mask()
    w1m = np.asarray(w1, np.float32) * mask
    w2m = np.asarray(w2, np.float32) * mask

    # lhsT weight layout: [ci_mod(p), tap, ci_chunk, co], bf16
    def wT(wm):
        wr = wm.reshape(C, 3, 128, KS, KS)          # [o, c, p, kh, kw]
        out = np.empty((128, NT, 3, C), np.float32)
        for t, (kh, kw) in enumerate(TAPS):
            out[:, t, :, :] = wr[:, :, :, kh, kw].transpose(2, 1, 0)
        return np.ascontiguousarray(out.astype(BF16NP))

    w1t_np, w2t_np = wT(w1m), wT(w2m)

    def chunked(v):  # [384] -> [128, 3]
        return np.ascontiguousarray(np.asarray(v, np.float32).reshape(3, 128).T)

    b1c, a1c = chunked(b1), chunked(a1)
    b2c, a2c = chunked(b2), chunked(a2)

    xq = x.reshape(B, 3, 128, H, W)
    in_maps = []
    for core in range(NCORES):
        b_, s = divmod(core, SPB)
        r0 = s * HS
        xs = np.zeros((128, NR, 3, WP), np.float32)
        lo, hi = r0 - 4, r0 - 4 + NR          # global rows [lo, hi)
        glo, ghi = max(lo, 0), min(hi, H)
        if ghi > glo:
            xs[:, glo - lo:ghi - lo, :, PAD:PAD + W] = \
                xq[b_, :, :, glo:ghi, :].transpose(1, 2, 0, 3)
        hmv = np.zeros((128, 2), np.float32) if s == 0 else np.ones((128, 2), np.float32)
        in_maps.append({
            "xs": xs.reshape(128, NR * 3, WP),
            "w1t": w1t_np, "w2t": w2t_np,
            "b1c": b1c, "a1c": a1c, "b2c": b2c, "a2c": a2c,
            "hm": hmv,
        })

    nc = _get_nc()
    kw = {}
    if _trace_dir is not None:
        kw = dict(trace=True, tmpdir=_trace_dir,
                  trace_cores=_trace_cores or [0])
    res = None
    for attempt in range(3):
        try:
            res = run_bass_kernel_spmd(nc, in_maps,
                                       core_ids=list(range(NCORES)), **kw)
            break
        except Exception:
            # transient NRT/axon device errors recover on retry
            if attempt == 2:
                raise
            import time
            time.sleep(5)

    y = np.empty_like(x)
    for core in range(NCORES):
        b_, s = divmod(core, SPB)
        r0 = s * HS
        ys = res.results[core]["ys"].reshape(128, HS, 3, W)
        y[b_, :, r0:r0 + HS, :] = ys.transpose(2, 0, 1, 3).reshape(C, HS, W)
    if _trace_dir is not None:
        return y, res
    return y



# revision 13
# speedup vs baseline: 1.2054x; 1.0112x over previous
"""Trainium2 Bass kernel for the EntropyResidualBlock (two masked 5x5 convs,
PReLU, residual) on 8 NeuronCores.

Sharding: 8 cores = 2 batches x 4 H-strips of 64 rows. Each core recomputes a
2-row y1 halo at the top of its strip (no cross-core communication); x halos
are host-zero-padded and a per-core halo-mask input zeroes the y1 halo rows
for strips at the image top (so conv2 sees correct zero padding).

The PixelCNN mask leaves only 13 of 25 taps nonzero (rows above center + left
of center + group-causal center), so each conv row is 13 taps x 3 ci-chunks x
3 co-chunks = 117 bf16 matmuls of [128,128]x[128,512] accumulated in PSUM.
conv1 -> conv2 are fused through rolling 4-row SBUF ring buffers.
"""

import os
import sys

import numpy as np
import ml_dtypes

for _p in ("/opt/trn_rl_repo",):
    if os.path.isdir(_p) and _p not in sys.path:
        sys.path.append(_p)

import concourse.bass as bass  # noqa: E402
import concourse.tile as tile  # noqa: E402
from concourse import bacc, mybir  # noqa: E402
from concourse.bass import ds  # noqa: E402
from concourse.bass_utils import run_bass_kernel_spmd  # noqa: E402

BF16NP = ml_dtypes.bfloat16
F32 = mybir.dt.float32
BF16 = mybir.dt.bfloat16
AF = mybir.ActivationFunctionType

B, C, H, W = 2, 384, 256, 512
NG, CPN, KS, PAD = 16, 24, 5, 2
NCORES = 8
SPB = 4            # strips per batch
HS = H // SPB      # 64 output rows per core
WP = 520           # padded row width in SBUF/DRAM (2 left pad + 512 + 6)
NR = HS + 5        # x rows staged per core: r0-4 .. r0+64 (last is prefetch slack)
TAPS = [(kh, kw) for kh in (0, 1) for kw in range(KS)] + [(2, 0), (2, 1), (2, 2)]
NT = len(TAPS)     # 13
NMM = NT * 3       # matmuls per psum accumulation group


def _build_mask() -> np.ndarray:
    m = np.zeros((C, C, KS, KS), np.float32)
    m[:, :, :PAD, :] = 1.0
    m[:, :, PAD, :PAD] = 1.0
    g = np.arange(C) // CPN
    m[:, :, PAD, PAD] = (g[None, :] <= g[:, None]).astype(np.float32)
    return m


def _build_nc():
    nc = bacc.Bacc("TRN2", target_bir_lowering=False, debug=False,
                   num_devices=NCORES)
    xs_d = nc.dram_tensor("xs", [128, NR * 3, WP], F32, kind="ExternalInput").ap()
    w1_d = nc.dram_tensor("w1t", [128, 3, NT, 3, 128], BF16,
                          kind="ExternalInput").ap()
    w2_d = nc.dram_tensor("w2t", [128, 3, NT, 3, 128], BF16,
                          kind="ExternalInput").ap()
    b1_d = nc.dram_tensor("b1c", [128, 3], F32, kind="ExternalInput").ap()
    a1_d = nc.dram_tensor("a1c", [128, 3], F32, kind="ExternalInput").ap()
    b2_d = nc.dram_tensor("b2c", [128, 3], F32, kind="ExternalInput").ap()
    a2_d = nc.dram_tensor("a2c", [128, 3], F32, kind="ExternalInput").ap()
    hm_d = nc.dram_tensor("hm", [128, 2], F32, kind="ExternalInput").ap()
    # rows 0,1 are dummies (garbage from the 2-row store lag); host reads 2..65
    ys_d = nc.dram_tensor("ys", [128, (HS + 2) * 3, W], F32,
                          kind="ExternalOutput").ap()

    with tile.TileContext(nc) as tc:
        with tc.tile_pool(name="wp", bufs=1) as wp, \
             tc.tile_pool(name="cp", bufs=1) as cp, \
             tc.tile_pool(name="ring", bufs=1) as rp, \
             tc.tile_pool(name="op", bufs=4) as op, \
             tc.tile_pool(name="pp", bufs=8, space="PSUM") as pp:

            w1t = wp.tile([128, 3, NT, 3, 128], BF16, name="w1sb", tag="w1sb")
            for _c in range(3):
                # coc-major: the first conv group can start after 1/3 arrives
                nc.gpsimd.dma_start(out=w1t[:, _c], in_=w1_d[:, _c])
            b1c = cp.tile([128, 3], F32, name="b1sb", tag="b1sb")
            nc.gpsimd.dma_start(out=b1c, in_=b1_d)
            a1c = cp.tile([128, 3], F32, name="a1sb", tag="a1sb")
            nc.gpsimd.dma_start(out=a1c, in_=a1_d)
            b2c = cp.tile([128, 3], F32, name="b2sb", tag="b2sb")
            nc.gpsimd.dma_start(out=b2c, in_=b2_d)
            a2c = cp.tile([128, 3], F32, name="a2sb", tag="a2sb")
            nc.gpsimd.dma_start(out=a2c, in_=a2_d)
            hm = cp.tile([128, 2], F32, name="hmsb", tag="hmsb")
            nc.gpsimd.dma_start(out=hm, in_=hm_d)

            xf = [rp.tile([128, 3, WP], F32, name=f"xf{j}", tag=f"xf{j}")
                  for j in range(4)]
            xb = [rp.tile([128, 3, WP], BF16, name=f"xb{j}", tag=f"xb{j}")
                  for j in range(4)]
            y1 = [rp.tile([128, 3, WP], BF16, name=f"y1r{j}", tag=f"y1r{j}")
                  for j in range(4)]
            for j in range(4):
                # W-pad columns of the y1 ring stay zero forever (conv2 zero pad)
                nc.vector.memset(y1[j][:, :, 0:PAD], 0.0)
                nc.vector.memset(y1[j][:, :, PAD + W:WP], 0.0)

            def load_x_row(row_expr, slot):
                # one DMA: [128, 3, WP] fp32 row (host zero-padded), then bf16 cast
                nc.sync.dma_start(out=xf[slot], in_=xs_d[:, ds(row_expr * 3, 3), :])
                nc.vector.tensor_copy(xb[slot], xf[slot])

            def conv_row(wt, ring, hmod, epilogue, dh0_last=False):
                taps = sorted(TAPS, key=lambda t: t[0] == 2) if dh0_last else TAPS
                for coc in range(3):
                    ps = pp.tile([128, W], F32, name="ps", tag="ps")
                    work = [(kh, kw, cic) for (kh, kw) in taps for cic in range(3)
                            # group-causal center tap: ci chunk 2 (groups>=10)
                            # never feeds co chunk 0 (groups<=5) - weights all 0
                            if not (kh == 2 and kw == 2 and cic == 2 and coc == 0)]
                    for n, (kh, kw, cic) in enumerate(work):
                        dh, dw = kh - 2, kw - 2
                        src = ring[(hmod + dh) % 4]
                        ti = TAPS.index((kh, kw))
                        nc.tensor.matmul(
                            ps,
                            lhsT=wt[:, coc, ti, cic, :],
                            rhs=src[:, cic, PAD + dw: PAD + dw + W],
                            start=(n == 0), stop=(n == len(work) - 1))
                    epilogue(coc, ps)

            def y1_epilogue(slot):
                def f(coc, ps):
                    nc.scalar.activation(
                        y1[slot][:, coc, PAD:PAD + W], ps, AF.Prelu,
                        bias=b1c[:, coc:coc + 1], scale=1.0,
                        alpha=a1c[:, coc:coc + 1])
                return f

            # ---- prologue: x[-4..-1], then y1[-2], y1[-1] (halo, maskable) ----
            for j in range(4):
                load_x_row(j, j)          # xs row j = x[r0-4+j] -> slot j
            w2t = wp.tile([128, 3, NT, 3, 128], BF16, name="w2sb", tag="w2sb")
            nc.gpsimd.dma_start(out=w2t, in_=w2_d)
            conv_row(w1t, xb, (-2) % 4, y1_epilogue((-2) % 4))
            nc.vector.tensor_scalar_mul(y1[(-2) % 4], y1[(-2) % 4], hm[:, 0:1])
            load_x_row(4, 0)              # x[0] -> slot 0
            conv_row(w1t, xb, (-1) % 4, y1_epilogue((-1) % 4))
            nc.vector.tensor_scalar_mul(y1[(-1) % 4], y1[(-1) % 4], hm[:, 1:2])

            # ---- main loop: 8 groups x 8 rows (ring slots = j%4, static) ----
            # Output stores lag their row by 2 positions so the back-edge
            # barrier never waits on a just-issued 786KB store; iteration 0
            # positions 0,1 store garbage into dummy ys rows 0,1.
            with tc.For_i(0, HS // 8, 1,
                          hint_engines=(mybir.EngineType.PE,)) as i:
                y2s_tiles = {j: op.tile([128, 3, W], F32, name=f"y2s{j}",
                                        tag="y2s") for j in range(8)}
                for j in range(8):
                    y2s = y2s_tiles[j]
                    # row h = 8*i + j; prefetch x[h+1] (xs row h+5)
                    load_x_row(i * 8 + (j + 5), (j + 1) % 4)
                    conv_row(w1t, xb, j % 4, y1_epilogue(j % 4))

                    def y2_epilogue(coc, ps, j=j, y2s=y2s):
                        nc.scalar.activation(
                            y2s[:, coc, :], ps, AF.Prelu,
                            bias=b2c[:, coc:coc + 1], scale=1.0,
                            alpha=a2c[:, coc:coc + 1])
                        nc.vector.tensor_add(
                            y2s[:, coc, :], y2s[:, coc, :],
                            xf[j % 4][:, coc, PAD:PAD + W])

                    conv_row(w2t, y1, j % 4, y2_epilogue, dh0_last=True)
                    # store row 8i+j-2 (tile from position (j+6)%8), at
                    # ys slot (8i+j-2)+2 = 8i+j
                    nc.sync.dma_start(out=ys_d[:, ds((i * 8 + j) * 3, 3), :],
                                      in_=y2s_tiles[(j + 6) % 8])
            # epilogue: rows 62,63 (tiles from positions 6,7) -> ys slots 64,65
            nc.sync.dma_start(out=ys_d[:, ds((HS) * 3, 3), :], in_=y2s_tiles[6])
            nc.sync.dma_start(out=ys_d[:, ds((HS + 1) * 3, 3), :],
                              in_=y2s_tiles[7])

    nc.compile()
    return nc


_NC_CACHE = {}


def _get_nc():
    if "nc" not in _NC_CACHE:
        _NC_CACHE["nc"] = _build_nc()
    return _NC_CACHE["nc"]


def kernel(x, w1, b1, a1, w2, b2, a2, _trace_dir=None, _trace_cores=None):
    x = np.asarray(x, np.float32)
    mask = _build_mask()
    w1m = np.asarray(w1, np.float32) * mask
    w2m = np.asarray(w2, np.float32) * mask

    # lhsT weight layout: [ci_mod(p), co_chunk, tap, ci_chunk, co_mod], bf16
    def wT(wm):
        wr = wm.reshape(3, 128, 3, 128, KS, KS)     # [oc, om, c, p, kh, kw]
        out = np.empty((128, 3, NT, 3, 128), np.float32)
        for t, (kh, kw) in enumerate(TAPS):
            out[:, :, t, :, :] = wr[:, :, :, :, kh, kw].transpose(3, 0, 2, 1)
        return np.ascontiguousarray(out.astype(BF16NP))

    w1t_np, w2t_np = wT(w1m), wT(w2m)

    def chunked(v):  # [384] -> [128, 3]
        return np.ascontiguousarray(np.asarray(v, np.float32).reshape(3, 128).T)

    b1c, a1c = chunked(b1), chunked(a1)
    b2c, a2c = chunked(b2), chunked(a2)

    xq = x.reshape(B, 3, 128, H, W)
    in_maps = []
    for core in range(NCORES):
        b_, s = divmod(core, SPB)
        r0 = s * HS
        xs = np.zeros((128, NR, 3, WP), np.float32)
        lo, hi = r0 - 4, r0 - 4 + NR          # global rows [lo, hi)
        glo, ghi = max(lo, 0), min(hi, H)
        if ghi > glo:
            xs[:, glo - lo:ghi - lo, :, PAD:PAD + W] = \
                xq[b_, :, :, glo:ghi, :].transpose(1, 2, 0, 3)
        hmv = np.zeros((128, 2), np.float32) if s == 0 else np.ones((128, 2), np.float32)
        in_maps.append({
            "xs": xs.reshape(128, NR * 3, WP),
            "w1t": w1t_np, "w2t": w2t_np,
            "b1c": b1c, "a1c": a1c, "b2c": b2c, "a2c": a2c,
            "hm": hmv,
        })

    nc = _get_nc()
    kw = {}
    if _trace_dir is not None:
        kw = dict(trace=True, tmpdir=_trace_dir,
                  trace_cores=_trace_cores or [0])
    res = None
    for attempt in range(3):
        try:
            res = run_bass_kernel_spmd(nc, in_maps,
                                       core_ids=list(range(NCORES)), **kw)
            break
        except Exception:
            # transient NRT/axon device errors recover on retry
            if attempt == 2:
                raise
            import time
            time.sleep(5)

    y = np.empty_like(x)
    for core in range(NCORES):
        b_, s = divmod(core, SPB)
        r0 = s * HS
        ys = res.results[core]["ys"].reshape(128, HS + 2, 3, W)[:, 2:]
        y[b_, :, r0:r0 + HS, :] = ys.transpose(2, 0, 1, 3).reshape(C, HS, W)
    if _trace_dir is not None:
        return y, res
    return y



# revision 21
# speedup vs baseline: 1.2131x; 1.0064x over previous
"""Trainium2 Bass kernel for the EntropyResidualBlock (two masked 5x5 convs,
PReLU, residual) on 8 NeuronCores.

Sharding: 8 cores = 2 batches x 4 H-strips of 64 rows. Each core recomputes a
2-row y1 halo at the top of its strip (no cross-core communication); x halos
are host-zero-padded and a per-core halo-mask input zeroes the y1 halo rows
for strips at the image top (so conv2 sees correct zero padding).

The PixelCNN mask leaves only 13 of 25 taps nonzero (rows above center + left
of center + group-causal center), so each conv row is 13 taps x 3 ci-chunks x
3 co-chunks = 117 bf16 matmuls of [128,128]x[128,512] accumulated in PSUM.
conv1 -> conv2 are fused through rolling 4-row SBUF ring buffers.
"""

import os
import sys

import numpy as np
import ml_dtypes

for _p in ("/opt/trn_rl_repo",):
    if os.path.isdir(_p) and _p not in sys.path:
        sys.path.append(_p)

import concourse.bass as bass  # noqa: E402
import concourse.tile as tile  # noqa: E402
from concourse import bacc, mybir  # noqa: E402
from concourse.bass import ds  # noqa: E402
from concourse.bass_utils import run_bass_kernel_spmd  # noqa: E402

BF16NP = ml_dtypes.bfloat16
F32 = mybir.dt.float32
BF16 = mybir.dt.bfloat16
AF = mybir.ActivationFunctionType

B, C, H, W = 2, 384, 256, 512
NG, CPN, KS, PAD = 16, 24, 5, 2
NCORES = 8
SPB = 4            # strips per batch
HS = H // SPB      # 64 output rows per core
WP = 520           # padded row width in SBUF/DRAM (2 left pad + 512 + 6)
NR = HS + 5        # x rows staged per core: r0-4 .. r0+64 (last is prefetch slack)
TAPS = [(kh, kw) for kh in (0, 1) for kw in range(KS)] + [(2, 0), (2, 1), (2, 2)]
NT = len(TAPS)     # 13
NMM = NT * 3       # matmuls per psum accumulation group


def _build_mask() -> np.ndarray:
    m = np.zeros((C, C, KS, KS), np.float32)
    m[:, :, :PAD, :] = 1.0
    m[:, :, PAD, :PAD] = 1.0
    g = np.arange(C) // CPN
    m[:, :, PAD, PAD] = (g[None, :] <= g[:, None]).astype(np.float32)
    return m


def _build_nc():
    nc = bacc.Bacc("TRN2", target_bir_lowering=False, debug=False,
                   num_devices=NCORES)
    xs_d = nc.dram_tensor("xs", [128, NR * 3, WP], F32, kind="ExternalInput").ap()
    w1_d = nc.dram_tensor("w1t", [128, 3, NT, 3, 128], BF16,
                          kind="ExternalInput").ap()
    w2_d = nc.dram_tensor("w2t", [128, 3, NT, 3, 128], BF16,
                          kind="ExternalInput").ap()
    b1_d = nc.dram_tensor("b1c", [128, 3], F32, kind="ExternalInput").ap()
    a1_d = nc.dram_tensor("a1c", [128, 3], F32, kind="ExternalInput").ap()
    b2_d = nc.dram_tensor("b2c", [128, 3], F32, kind="ExternalInput").ap()
    a2_d = nc.dram_tensor("a2c", [128, 3], F32, kind="ExternalInput").ap()
    hm_d = nc.dram_tensor("hm", [128, 2], F32, kind="ExternalInput").ap()
    # rows 0,1 are dummies (garbage from the 2-row store lag); host reads 2..65
    ys_d = nc.dram_tensor("ys", [128, (HS + 2) * 3, W], F32,
                          kind="ExternalOutput").ap()

    with tile.TileContext(nc) as tc:
        with tc.tile_pool(name="wp", bufs=1) as wp, \
             tc.tile_pool(name="cp", bufs=1) as cp, \
             tc.tile_pool(name="ring", bufs=1) as rp, \
             tc.tile_pool(name="op", bufs=4) as op, \
             tc.tile_pool(name="pp", bufs=8, space="PSUM") as pp:

            w1t = wp.tile([128, 3, NT, 3, 128], BF16, name="w1sb", tag="w1sb")
            for _c in range(3):
                # coc-major: the first conv group can start after 1/3 arrives
                nc.gpsimd.dma_start(out=w1t[:, _c], in_=w1_d[:, _c])
            b1c = cp.tile([128, 3], F32, name="b1sb", tag="b1sb")
            nc.gpsimd.dma_start(out=b1c, in_=b1_d)
            a1c = cp.tile([128, 3], F32, name="a1sb", tag="a1sb")
            nc.gpsimd.dma_start(out=a1c, in_=a1_d)
            b2c = cp.tile([128, 3], F32, name="b2sb", tag="b2sb")
            nc.gpsimd.dma_start(out=b2c, in_=b2_d)
            a2c = cp.tile([128, 3], F32, name="a2sb", tag="a2sb")
            nc.gpsimd.dma_start(out=a2c, in_=a2_d)
            hm = cp.tile([128, 2], F32, name="hmsb", tag="hmsb")
            nc.gpsimd.dma_start(out=hm, in_=hm_d)

            xf = [rp.tile([128, 3, WP], F32, name=f"xf{j}", tag=f"xf{j}")
                  for j in range(4)]
            xb = [rp.tile([128, 3, WP], BF16, name=f"xb{j}", tag=f"xb{j}")
                  for j in range(4)]
            y1 = [rp.tile([128, 3, WP], BF16, name=f"y1r{j}", tag=f"y1r{j}")
                  for j in range(4)]
            for j in range(4):
                # W-pad columns of the y1 ring stay zero forever (conv2 zero pad)
                nc.vector.memset(y1[j][:, :, 0:PAD], 0.0)
                nc.vector.memset(y1[j][:, :, PAD + W:WP], 0.0)

            def load_x_row(row_expr, slot):
                # one DMA: [128, 3, WP] fp32 row (host zero-padded), then bf16 cast
                nc.sync.dma_start(out=xf[slot], in_=xs_d[:, ds(row_expr * 3, 3), :])
                nc.vector.tensor_copy(xb[slot], xf[slot])

            def conv_row(wt, ring, hmod, epilogue, dh0_last=False):
                taps = sorted(TAPS, key=lambda t: t[0] == 2) if dh0_last else TAPS
                for coc in range(3):
                    ps = pp.tile([128, W], F32, name="ps", tag="ps")
                    work = [(kh, kw, cic) for (kh, kw) in taps for cic in range(3)
                            # group-causal center tap: ci chunk 2 (groups>=10)
                            # never feeds co chunk 0 (groups<=5) - weights all 0
                            if not (kh == 2 and kw == 2 and cic == 2 and coc == 0)]
                    for n, (kh, kw, cic) in enumerate(work):
                        dh, dw = kh - 2, kw - 2
                        src = ring[(hmod + dh) % 4]
                        ti = TAPS.index((kh, kw))
                        nc.tensor.matmul(
                            ps,
                            lhsT=wt[:, coc, ti, cic, :],
                            rhs=src[:, cic, PAD + dw: PAD + dw + W],
                            start=(n == 0), stop=(n == len(work) - 1))
                    epilogue(coc, ps)

            def y1_epilogue(slot):
                def f(coc, ps):
                    nc.scalar.activation(
                        y1[slot][:, coc, PAD:PAD + W], ps, AF.Prelu,
                        bias=b1c[:, coc:coc + 1], scale=1.0,
                        alpha=a1c[:, coc:coc + 1])
                return f

            # ---- prologue: x[-4..-1], then y1[-2], y1[-1] (halo, maskable) ----
            for j in range(4):
                load_x_row(j, j)          # xs row j = x[r0-4+j] -> slot j
            w2t = wp.tile([128, 3, NT, 3, 128], BF16, name="w2sb", tag="w2sb")
            nc.gpsimd.dma_start(out=w2t, in_=w2_d)
            conv_row(w1t, xb, (-2) % 4, y1_epilogue((-2) % 4))
            nc.vector.tensor_scalar_mul(y1[(-2) % 4], y1[(-2) % 4], hm[:, 0:1])
            load_x_row(4, 0)              # x[0] -> slot 0
            conv_row(w1t, xb, (-1) % 4, y1_epilogue((-1) % 4))
            nc.vector.tensor_scalar_mul(y1[(-1) % 4], y1[(-1) % 4], hm[:, 1:2])

            # ---- main loop: 8 groups x 8 rows (ring slots = j%4, static) ----
            # Output stores lag their row by 2 positions so the back-edge
            # barrier never waits on a just-issued 786KB store; iteration 0
            # positions 0,1 store garbage into dummy ys rows 0,1.
            with tc.For_i(0, HS // 32, 1,
                          hint_engines=(mybir.EngineType.PE,)) as i:
                y2s_tiles = {j: op.tile([128, 3, W], F32, name=f"y2s{j}",
                                        tag="y2s") for j in range(32)}
                for j in range(32):
                    y2s = y2s_tiles[j]
                    # row h = 8*i + j; prefetch x[h+1] (xs row h+5)
                    load_x_row(i * 32 + (j + 5), (j + 1) % 4)
                    conv_row(w1t, xb, j % 4, y1_epilogue(j % 4))

                    def y2_epilogue(coc, ps, j=j, y2s=y2s):
                        nc.scalar.activation(
                            y2s[:, coc, :], ps, AF.Prelu,
                            bias=b2c[:, coc:coc + 1], scale=1.0,
                            alpha=a2c[:, coc:coc + 1])
                        nc.vector.tensor_add(
                            y2s[:, coc, :], y2s[:, coc, :],
                            xf[j % 4][:, coc, PAD:PAD + W])

                    conv_row(w2t, y1, j % 4, y2_epilogue, dh0_last=True)
                    # store row 8i+j-2 (tile from position (j+6)%8), at
                    # ys slot (8i+j-2)+2 = 8i+j
                    nc.sync.dma_start(out=ys_d[:, ds((i * 32 + j) * 3, 3), :],
                                      in_=y2s_tiles[(j + 30) % 32])
            # epilogue: rows 62,63 (tiles from positions 6,7) -> ys slots 64,65
            nc.sync.dma_start(out=ys_d[:, ds((HS) * 3, 3), :],
                              in_=y2s_tiles[30])
            nc.sync.dma_start(out=ys_d[:, ds((HS + 1) * 3, 3), :],
                              in_=y2s_tiles[31])

    nc.compile()
    return nc


_NC_CACHE = {}


def _get_nc():
    if "nc" not in _NC_CACHE:
        _NC_CACHE["nc"] = _build_nc()
    return _NC_CACHE["nc"]


def kernel(x, w1, b1, a1, w2, b2, a2, _trace_dir=None, _trace_cores=None):
    x = np.asarray(x, np.float32)
    mask = _build_mask()
    w1m = np.asarray(w1, np.float32) * mask
    w2m = np.asarray(w2, np.float32) * mask

    # lhsT weight layout: [ci_mod(p), co_chunk, tap, ci_chunk, co_mod], bf16
    def wT(wm):
        wr = wm.reshape(3, 128, 3, 128, KS, KS)     # [oc, om, c, p, kh, kw]
        out = np.empty((128, 3, NT, 3, 128), np.float32)
        for t, (kh, kw) in enumerate(TAPS):
            out[:, :, t, :, :] = wr[:, :, :, :, kh, kw].transpose(3, 0, 2, 1)
        return np.ascontiguousarray(out.astype(BF16NP))

    w1t_np, w2t_np = wT(w1m), wT(w2m)

    def chunked(v):  # [384] -> [128, 3]
        return np.ascontiguousarray(np.asarray(v, np.float32).reshape(3, 128).T)

    b1c, a1c = chunked(b1), chunked(a1)
    b2c, a2c = chunked(b2), chunked(a2)

    xq = x.reshape(B, 3, 128, H, W)
    in_maps = []
    for core in range(NCORES):
        b_, s = divmod(core, SPB)
        r0 = s * HS
        xs = np.zeros((128, NR, 3, WP), np.float32)
        lo, hi = r0 - 4, r0 - 4 + NR          # global rows [lo, hi)
        glo, ghi = max(lo, 0), min(hi, H)
        if ghi > glo:
            xs[:, glo - lo:ghi - lo, :, PAD:PAD + W] = \
                xq[b_, :, :, glo:ghi, :].transpose(1, 2, 0, 3)
        hmv = np.zeros((128, 2), np.float32) if s == 0 else np.ones((128, 2), np.float32)
        in_maps.append({
            "xs": xs.reshape(128, NR * 3, WP),
            "w1t": w1t_np, "w2t": w2t_np,
            "b1c": b1c, "a1c": a1c, "b2c": b2c, "a2c": a2c,
            "hm": hmv,
        })

    nc = _get_nc()
    kw = {}
    if _trace_dir is not None:
        kw = dict(trace=True, tmpdir=_trace_dir,
                  trace_cores=_trace_cores or [0])
    res = None
    for attempt in range(3):
        try:
            res = run_bass_kernel_spmd(nc, in_maps,
                                       core_ids=list(range(NCORES)), **kw)
            break
        except Exception:
            # transient NRT/axon device errors recover on retry
            if attempt == 2:
                raise
            import time
            time.sleep(5)

    y = np.empty_like(x)
    for core in range(NCORES):
        b_, s = divmod(core, SPB)
        r0 = s * HS
        ys = res.results[core]["ys"].reshape(128, HS + 2, 3, W)[:, 2:]
        y[b_, :, r0:r0 + HS, :] = ys.transpose(2, 0, 1, 3).reshape(C, HS, W)
    if _trace_dir is not None:
        return y, res
    return y

